# revision 26
# baseline (speedup 1.0000x reference)
"""Trainium2 Bass kernel for a channel-attention block.

Reference math (per batch sample, a: [C, N] with C=128 channels,
N = H*W spatial):
    b   = a @ a.T                  # [C, C] channel affinity (Gram)
    x   = softmax(b, axis=-1)
    c   = x @ a                    # [C, N]
    out = beta * c + a

Sharding: data-parallel over the batch dim — 16 samples / 8 cores =
2 samples per NeuronCore, no cross-core communication.

Single-HBM-pass design (per sample):
  stage A: SWDGE (gpsimd) cast-DMA loads `a` in [128, LW] tiles,
           converting f32 -> bf16 in flight; the bf16 tiles stay
           RESIDENT in SBUF (16 MB/sample) so `a` is read from HBM
           exactly once and no compute engine spends time casting.
           Each tile is PE-transposed in 128-col blocks into PSUM,
           copied back to SBUF (DVE), and Gram-accumulated into one
           PSUM bank via bf16 matmuls.
  stage B: row softmax on b (DVE max, ACT exp(+bias) with fused row
           sum, DVE reciprocal). The whole affine epilogue folds into
           the stage-C weights: W = (beta/rowsum) * E + I, so
           W @ a = beta*softmax(b)@a + a IS the output — no add pass.
  stage C: c_ps = W.T.T @ a_bf16 straight from the SBUF-resident tiles
           (no second HBM read); the epilogue is a pure PSUM->SBUF
           bf16 copy, alternated between DVE and ACT; stored to HBM as
           bf16 (host upcasts to f32).

HBM traffic per core: 64 MB read (f32 a, once) + 32 MB write (bf16
out) = 96 MB. Stage C of sample s is emission-interleaved with stage A
of sample s+1 (C runs `lead` tiles ahead; SWDGE loads run `prefetch`
tiles ahead; `holdback` C tiles are re-emitted after softmax(s+1) so
PE has queued work through the phase transition). The c_ps pool keeps
4 PSUM tiles in flight so the DVE/ACT output copies pipeline instead
of serializing behind matmuls.
"""

import numpy as np

import concourse.bass as bass
import concourse.mybir as mybir
import concourse.tile as tile
from concourse import bacc
from concourse.bass_utils import run_bass_kernel_spmd
from concourse.masks import make_identity

F32 = mybir.dt.float32
BF16 = mybir.dt.bfloat16

N_CORES = 8
B, C, H, W = 16, 128, 256, 256
N_FULL = H * W
S = B // N_CORES  # samples per core


def build(S=S, C=C, N=N_FULL, LW=4096, TW=1024, MM_N=512, cache_extra=3,
          lead=3, prefetch=3, holdback=2, out_dt="bf16", eng_atcopy="dve",
          eng_cast="act", cast_split=False, ld_mode="headsplit",
          stage_bufs=1, tp_bufs=3, gram_bufs=1, cps_bufs=4,
          at_bufs=4, cout_bufs=3):
    """Build + compile the per-core Bass program."""
    assert C == 128 and N % LW == 0 and LW % TW == 0 and TW % 128 == 0
    assert LW % MM_N == 0 and MM_N % 512 == 0
    assert prefetch <= lead + 1 and prefetch <= cache_extra
    nc = bacc.Bacc("TRN2", target_bir_lowering=False, debug=False)

    a_d = nc.dram_tensor("a", [S, C, N], F32, kind="ExternalInput").ap()
    beta_d = nc.dram_tensor("beta", [C, 1], F32, kind="ExternalInput").ap()
    o_dt = BF16 if out_dt == "bf16" else F32
    out_d = nc.dram_tensor("out", [S, C, N], o_dt, kind="ExternalOutput").ap()

    n_loads = N // LW
    n_chunks = LW // MM_N
    n_gram_mm = N // 128

    with tile.TileContext(nc) as tc:
        with (
            tc.tile_pool(name="const", bufs=1) as const_pool,
            tc.tile_pool(name="acache", bufs=n_loads + cache_extra) as cache_pool,
            tc.tile_pool(name="stage", bufs=stage_bufs) as stage_pool,
            tc.tile_pool(name="at", bufs=at_bufs) as at_pool,
            tc.tile_pool(name="sm", bufs=2) as sm_pool,
            tc.tile_pool(name="cout", bufs=cout_bufs) as cout_pool,
            tc.tile_pool(name="tp_ps", bufs=tp_bufs, space="PSUM") as tp_psum,
            tc.tile_pool(name="gram_ps", bufs=gram_bufs, space="PSUM") as gram_psum,
            tc.tile_pool(name="c_ps", bufs=cps_bufs, space="PSUM") as c_psum,
        ):
            ident_bf = const_pool.tile([128, 128], BF16, tag="identbf")
            make_identity(nc, ident_bf)
            beta_sb = const_pool.tile([C, 1], F32, tag="beta")
            nc.sync.dma_start(beta_sb, beta_d)

            def copy_op(engine_sel, idx, out, in_):
                """Route a copy/cast to ACT or DVE per engine_sel."""
                if engine_sel == "act" or (engine_sel == "alt" and idx % 2 == 0):
                    nc.scalar.copy(out, in_)
                else:
                    nc.vector.tensor_copy(out, in_)

            gram_state = {}   # s -> [b_ps, mm_count]
            xt_w = {}         # s -> beta-scaled lhsT weights for stage C
            cached = {}       # (s, j) -> SBUF-resident bf16 a tile

            def load_cast(s, j):
                """HWDGE f32 load into staging; ACT/DVE cast into cache
                (or SWDGE cast-in-DMA when ld_mode='swdge')."""
                abf = cache_pool.tile([C, LW], BF16, tag="acache",
                                      name=f"ac_{s}_{j}")
                cached[(s, j)] = abf
                src = a_d[s, :, j * LW:(j + 1) * LW]
                # headsplit: in sample 0 (the head phase) the HWDGE rings
                # and ACT are otherwise idle, and SWDGE cast-DMA alone paces
                # the head at half line-rate. So odd head tiles load f32 via
                # HWDGE + ACT cast while even tiles use SWDGE — the two paths
                # split the head's DMA time. Every later sample is SWDGE-only
                # (zero engine cost, keeps ACT/DVE free for the epilogue).
                use_swdge = (
                    ld_mode == "swdge"
                    or (ld_mode == "hybrid" and s > 0)
                    or (ld_mode == "headsplit" and (s > 0 or j % 2 == 0))
                )
                if use_swdge:
                    nc.gpsimd.dma_start(abf, src)
                    return
                stg = stage_pool.tile([C, LW], F32, tag="stage",
                                      name=f"stg_{s}_{j}")
                ld = nc.sync if j % 4 == 1 else nc.scalar
                ld.dma_start(stg, src)
                if cast_split:
                    hw = LW // 2
                    nc.scalar.copy(abf[:, :hw], stg[:, :hw])
                    nc.vector.tensor_copy(abf[:, hw:], stg[:, hw:])
                else:
                    copy_op(eng_cast, j, abf, stg)

            def tg_chunk(s, j):
                """PE-transpose tile j of sample s and Gram-accumulate."""
                if s not in gram_state:
                    b_ps = gram_psum.tile([C, C], F32, tag="gram",
                                          name=f"gram_{s}")
                    gram_state[s] = [b_ps, 0]
                st = gram_state[s]
                b_ps = st[0]
                abf = cached[(s, j)]
                for g in range(LW // TW):
                    tp = tp_psum.tile([128, TW], BF16, tag="tp",
                                      name=f"tp_{s}_{j}_{g}")
                    for q in range(TW // 128):
                        nc.tensor.transpose(
                            tp[:, q * 128:(q + 1) * 128],
                            abf[:, g * TW + q * 128:g * TW + (q + 1) * 128],
                            ident_bf,
                        )
                    at_t = at_pool.tile([128, TW], BF16, tag="at",
                                        name=f"at_{s}_{j}_{g}")
                    copy_op(eng_atcopy, g, at_t, tp)
                    for q in range(TW // 128):
                        st[1] += 1
                        nc.tensor.matmul(
                            b_ps,
                            lhsT=at_t[:, q * 128:(q + 1) * 128],
                            rhs=at_t[:, q * 128:(q + 1) * 128],
                            start=(st[1] == 1),
                            stop=(st[1] == n_gram_mm),
                        )

            def softmax(s):
                """Fold the whole affine epilogue into the stage-C
                weights: W = (beta/rowsum) * exp(b - rowmax) + I, so
                W @ a = beta*softmax(b)@a + a = out."""
                b_ps = gram_state[s][0]
                negm = sm_pool.tile([C, 1], F32, tag="negm", name=f"negm_{s}")
                nc.vector.tensor_reduce(
                    negm, b_ps, axis=mybir.AxisListType.X,
                    op=mybir.AluOpType.max, negate=True,
                )
                e_t = sm_pool.tile([C, C], BF16, tag="e", name=f"e_{s}")
                ssum = sm_pool.tile([C, 1], F32, tag="ssum", name=f"ssum_{s}")
                nc.scalar.activation(
                    e_t, b_ps, mybir.ActivationFunctionType.Exp,
                    bias=negm, accum_out=ssum,
                )
                rec = sm_pool.tile([C, 1], F32, tag="rec", name=f"rec_{s}")
                nc.vector.reciprocal(rec, ssum)
                bs = sm_pool.tile([C, 1], F32, tag="bs", name=f"bs_{s}")
                nc.vector.tensor_scalar_mul(bs, rec, beta_sb)
                e_s = sm_pool.tile([C, C], BF16, tag="es", name=f"es_{s}")
                nc.vector.tensor_scalar_mul(e_s, e_t, bs)
                e_w = sm_pool.tile([C, C], BF16, tag="ew", name=f"ew_{s}")
                nc.vector.tensor_tensor(
                    out=e_w, in0=e_s, in1=ident_bf, op=mybir.AluOpType.add,
                )
                xt_ps = tp_psum.tile([128, TW], BF16, tag="tp", name=f"xtp_{s}")
                nc.tensor.transpose(xt_ps[:, :128], e_w, ident_bf)
                xt_sb = sm_pool.tile([C, C], BF16, tag="xt", name=f"xt_{s}")
                nc.scalar.copy(xt_sb, xt_ps[:, :128])
                xt_w[s] = xt_sb

            def stage_c_chunk(s, j):
                """c_ps = W @ a_bf16 from SBUF (the full output); the
                epilogue is a pure PSUM->SBUF bf16 copy (DVE/ACT alt)."""
                abf = cached.pop((s, j))
                obf = cout_pool.tile([C, LW], o_dt, tag="cout",
                                     name=f"cout_{s}_{j}")
                for q in range(n_chunks):
                    c_ps = c_psum.tile([128, MM_N], F32, tag="cps",
                                       name=f"cps_{s}_{j}_{q}")
                    for h in range(MM_N // 512):
                        sl = slice(q * MM_N + h * 512, q * MM_N + (h + 1) * 512)
                        nc.tensor.matmul(
                            c_ps[:, h * 512:(h + 1) * 512],
                            lhsT=xt_w[s], rhs=abf[:, sl],
                            start=True, stop=True,
                        )
                    osl = slice(q * MM_N, (q + 1) * MM_N)
                    copy_op("alt", q + (j % 2), obf[:, osl], c_ps)
                stq = nc.scalar if j % 2 == 0 else nc.sync
                stq.dma_start(out_d[s, :, j * LW:(j + 1) * LW], obf)

            # ── Emission schedule ──────────────────────────────────────
            for j in range(n_loads):
                load_cast(0, j)
                tg_chunk(0, j)
            for j in range(min(prefetch, n_loads)):
                load_cast(1, j) if S > 1 else None
            softmax(0)
            for s in range(1, S):
                ld = min(lead, n_loads)
                hb = min(holdback, n_loads - ld) if n_loads > ld else 0
                for j in range(ld):
                    stage_c_chunk(s - 1, j)
                for j in range(n_loads):
                    if j + ld < n_loads - hb:
                        stage_c_chunk(s - 1, j + ld)
                    tg_chunk(s, j)
                    if j + prefetch < n_loads:
                        load_cast(s, j + prefetch)
                if s + 1 < S:
                    for j in range(min(prefetch, n_loads)):
                        load_cast(s + 1, j)
                softmax(s)
                # Held-back C(s-1) tiles give PE work to chew on while the
                # softmax(s) DVE/ACT chain resolves (kills the phase dip).
                for j in range(n_loads - hb, n_loads):
                    stage_c_chunk(s - 1, j)
            for j in range(n_loads):
                stage_c_chunk(S - 1, j)

    nc.compile()
    return nc


_NC_CACHE: dict = {}


def _get_nc(**kw):
    key = tuple(sorted(kw.items()))
    if key not in _NC_CACHE:
        _NC_CACHE[key] = build(**kw)
    return _NC_CACHE[key]


def kernel(a, beta):
    """Full-input entry point: a [16,128,256,256] f32, beta [1] f32."""
    a = np.ascontiguousarray(np.asarray(a, dtype=np.float32))
    beta = np.asarray(beta, dtype=np.float32)
    nb, ch, h, w = a.shape
    n = h * w
    s = nb // N_CORES
    a3 = a.reshape(nb, ch, n)
    beta_b = np.broadcast_to(beta.reshape(1, 1), (ch, 1)).copy()

    nc = _get_nc(S=s, C=ch, N=n)
    in_maps = [
        {"a": a3[i * s:(i + 1) * s], "beta": beta_b} for i in range(N_CORES)
    ]
    res = run_bass_kernel_spmd(nc, in_maps, list(range(N_CORES)))
    out = np.concatenate(
        [np.asarray(res.results[i]["out"]) for i in range(N_CORES)], axis=0
    )
    return out.reshape(nb, ch, h, w).astype(np.float32)


# revision 27
# speedup vs baseline: 1.0628x; 1.0628x over previous
"""Trainium2 Bass kernel for a channel-attention block.

Reference math (per batch sample, a: [C, N] with C=128 channels,
N = H*W spatial):
    b   = a @ a.T                  # [C, C] channel affinity (Gram)
    x   = softmax(b, axis=-1)
    c   = x @ a                    # [C, N]
    out = beta * c + a

Sharding: data-parallel over the batch dim — 16 samples / 8 cores =
2 samples per NeuronCore, no cross-core communication.

Single-HBM-pass design (per sample):
  stage A: SWDGE (gpsimd) cast-DMA loads `a` in [128, LW] tiles,
           converting f32 -> bf16 in flight; the bf16 tiles stay
           RESIDENT in SBUF (16 MB/sample) so `a` is read from HBM
           exactly once and no compute engine spends time casting.
           Each tile is PE-transposed in 128-col blocks into PSUM,
           copied back to SBUF (DVE), and Gram-accumulated into one
           PSUM bank via bf16 matmuls.
  stage B: row softmax on b (DVE max, ACT exp(+bias) with fused row
           sum, DVE reciprocal). The whole affine epilogue folds into
           the stage-C weights: W = (beta/rowsum) * E + I, so
           W @ a = beta*softmax(b)@a + a IS the output — no add pass.
  stage C: c_ps = W.T.T @ a_bf16 straight from the SBUF-resident tiles
           (no second HBM read); the epilogue is a pure PSUM->SBUF
           bf16 copy, alternated between DVE and ACT; stored to HBM as
           bf16 (host upcasts to f32).

HBM traffic per core: 64 MB read (f32 a, once) + 32 MB write (bf16
out) = 96 MB. Stage C of sample s is emission-interleaved with stage A
of sample s+1 (C runs `lead` tiles ahead; SWDGE loads run `prefetch`
tiles ahead; `holdback` C tiles are re-emitted after softmax(s+1) so
PE has queued work through the phase transition). The c_ps pool keeps
4 PSUM tiles in flight so the DVE/ACT output copies pipeline instead
of serializing behind matmuls.
"""

import numpy as np

import concourse.bass as bass
import concourse.mybir as mybir
import concourse.tile as tile
from concourse import bacc
from concourse.bass_utils import run_bass_kernel_spmd
from concourse.masks import make_identity

F32 = mybir.dt.float32
BF16 = mybir.dt.bfloat16

N_CORES = 8
B, C, H, W = 16, 128, 256, 256
N_FULL = H * W
S = B // N_CORES  # samples per core


def build(S=S, C=C, N=N_FULL, LW=4096, TW=1024, MM_N=512, cache_extra=5,
          lead=4, prefetch=5, holdback=3, out_dt="bf16", eng_atcopy="dve",
          eng_cast="act", cast_split=False, ld_mode="swdge",
          stage_bufs=2, tp_bufs=3, gram_bufs=1, cps_bufs=4,
          at_bufs=4, cout_bufs=3):
    """Build + compile the per-core Bass program."""
    assert C == 128 and N % LW == 0 and LW % TW == 0 and TW % 128 == 0
    assert LW % MM_N == 0 and MM_N % 512 == 0
    assert prefetch <= lead + 1 and prefetch <= cache_extra
    nc = bacc.Bacc("TRN2", target_bir_lowering=False, debug=False)

    a_d = nc.dram_tensor("a", [S, C, N], F32, kind="ExternalInput").ap()
    beta_d = nc.dram_tensor("beta", [C, 1], F32, kind="ExternalInput").ap()
    o_dt = BF16 if out_dt == "bf16" else F32
    out_d = nc.dram_tensor("out", [S, C, N], o_dt, kind="ExternalOutput").ap()

    n_loads = N // LW
    n_chunks = LW // MM_N
    n_gram_mm = N // 128

    with tile.TileContext(nc) as tc:
        with (
            tc.tile_pool(name="const", bufs=1) as const_pool,
            tc.tile_pool(name="acache", bufs=n_loads + cache_extra) as cache_pool,
            tc.tile_pool(name="stage", bufs=stage_bufs) as stage_pool,
            tc.tile_pool(name="at", bufs=at_bufs) as at_pool,
            tc.tile_pool(name="sm", bufs=2) as sm_pool,
            tc.tile_pool(name="cout", bufs=cout_bufs) as cout_pool,
            tc.tile_pool(name="tp_ps", bufs=tp_bufs, space="PSUM") as tp_psum,
            tc.tile_pool(name="gram_ps", bufs=gram_bufs, space="PSUM") as gram_psum,
            tc.tile_pool(name="c_ps", bufs=cps_bufs, space="PSUM") as c_psum,
        ):
            ident_bf = const_pool.tile([128, 128], BF16, tag="identbf")
            make_identity(nc, ident_bf)
            beta_sb = const_pool.tile([C, 1], F32, tag="beta")
            nc.sync.dma_start(beta_sb, beta_d)

            def copy_op(engine_sel, idx, out, in_):
                """Route a copy/cast to ACT or DVE per engine_sel."""
                if engine_sel == "act" or (engine_sel == "alt" and idx % 2 == 0):
                    nc.scalar.copy(out, in_)
                else:
                    nc.vector.tensor_copy(out, in_)

            gram_state = {}   # s -> [b_ps, mm_count]
            xt_w = {}         # s -> beta-scaled lhsT weights for stage C
            cached = {}       # (s, j) -> SBUF-resident bf16 a tile

            def load_cast(s, j):
                """HWDGE f32 load into staging; ACT/DVE cast into cache
                (or SWDGE cast-in-DMA when ld_mode='swdge')."""
                abf = cache_pool.tile([C, LW], BF16, tag="acache",
                                      name=f"ac_{s}_{j}")
                cached[(s, j)] = abf
                src = a_d[s, :, j * LW:(j + 1) * LW]
                # headsplit: in sample 0 (the head phase) the HWDGE rings
                # and ACT are otherwise idle, and SWDGE cast-DMA alone paces
                # the head at half line-rate. So odd head tiles load f32 via
                # HWDGE + ACT cast while even tiles use SWDGE — the two paths
                # split the head's DMA time. Every later sample is SWDGE-only
                # (zero engine cost, keeps ACT/DVE free for the epilogue).
                use_swdge = (
                    ld_mode == "swdge"
                    or (ld_mode == "hybrid" and s > 0)
                    or (ld_mode == "headsplit" and (s > 0 or j % 2 == 0))
                )
                if use_swdge:
                    nc.gpsimd.dma_start(abf, src)
                    return
                stg = stage_pool.tile([C, LW], F32, tag="stage",
                                      name=f"stg_{s}_{j}")
                ld = nc.sync if j % 4 == 1 else nc.scalar
                ld.dma_start(stg, src)
                if cast_split:
                    hw = LW // 2
                    nc.scalar.copy(abf[:, :hw], stg[:, :hw])
                    nc.vector.tensor_copy(abf[:, hw:], stg[:, hw:])
                else:
                    copy_op(eng_cast, j, abf, stg)

            def tg_chunk(s, j):
                """PE-transpose tile j of sample s and Gram-accumulate."""
                if s not in gram_state:
                    b_ps = gram_psum.tile([C, C], F32, tag="gram",
                                          name=f"gram_{s}")
                    gram_state[s] = [b_ps, 0]
                st = gram_state[s]
                b_ps = st[0]
                abf = cached[(s, j)]
                for g in range(LW // TW):
                    tp = tp_psum.tile([128, TW], BF16, tag="tp",
                                      name=f"tp_{s}_{j}_{g}")
                    for q in range(TW // 128):
                        nc.tensor.transpose(
                            tp[:, q * 128:(q + 1) * 128],
                            abf[:, g * TW + q * 128:g * TW + (q + 1) * 128],
                            ident_bf,
                        )
                    at_t = at_pool.tile([128, TW], BF16, tag="at",
                                        name=f"at_{s}_{j}_{g}")
                    copy_op(eng_atcopy, g, at_t, tp)
                    for q in range(TW // 128):
                        st[1] += 1
                        nc.tensor.matmul(
                            b_ps,
                            lhsT=at_t[:, q * 128:(q + 1) * 128],
                            rhs=at_t[:, q * 128:(q + 1) * 128],
                            start=(st[1] == 1),
                            stop=(st[1] == n_gram_mm),
                        )

            def softmax(s):
                """Fold the whole affine epilogue into the stage-C
                weights: W = (beta/rowsum) * exp(b - rowmax) + I, so
                W @ a = beta*softmax(b)@a + a = out."""
                b_ps = gram_state[s][0]
                negm = sm_pool.tile([C, 1], F32, tag="negm", name=f"negm_{s}")
                nc.vector.tensor_reduce(
                    negm, b_ps, axis=mybir.AxisListType.X,
                    op=mybir.AluOpType.max, negate=True,
                )
                e_t = sm_pool.tile([C, C], BF16, tag="e", name=f"e_{s}")
                ssum = sm_pool.tile([C, 1], F32, tag="ssum", name=f"ssum_{s}")
                nc.scalar.activation(
                    e_t, b_ps, mybir.ActivationFunctionType.Exp,
                    bias=negm, accum_out=ssum,
                )
                rec = sm_pool.tile([C, 1], F32, tag="rec", name=f"rec_{s}")
                nc.vector.reciprocal(rec, ssum)
                bs = sm_pool.tile([C, 1], F32, tag="bs", name=f"bs_{s}")
                nc.vector.tensor_scalar_mul(bs, rec, beta_sb)
                e_s = sm_pool.tile([C, C], BF16, tag="es", name=f"es_{s}")
                nc.vector.tensor_scalar_mul(e_s, e_t, bs)
                e_w = sm_pool.tile([C, C], BF16, tag="ew", name=f"ew_{s}")
                nc.vector.tensor_tensor(
                    out=e_w, in0=e_s, in1=ident_bf, op=mybir.AluOpType.add,
                )
                xt_ps = tp_psum.tile([128, TW], BF16, tag="tp", name=f"xtp_{s}")
                nc.tensor.transpose(xt_ps[:, :128], e_w, ident_bf)
                xt_sb = sm_pool.tile([C, C], BF16, tag="xt", name=f"xt_{s}")
                nc.scalar.copy(xt_sb, xt_ps[:, :128])
                xt_w[s] = xt_sb

            def stage_c_chunk(s, j):
                """c_ps = W @ a_bf16 from SBUF (the full output); the
                epilogue is a pure PSUM->SBUF bf16 copy (DVE/ACT alt)."""
                abf = cached.pop((s, j))
                obf = cout_pool.tile([C, LW], o_dt, tag="cout",
                                     name=f"cout_{s}_{j}")
                for q in range(n_chunks):
                    c_ps = c_psum.tile([128, MM_N], F32, tag="cps",
                                       name=f"cps_{s}_{j}_{q}")
                    for h in range(MM_N // 512):
                        sl = slice(q * MM_N + h * 512, q * MM_N + (h + 1) * 512)
                        nc.tensor.matmul(
                            c_ps[:, h * 512:(h + 1) * 512],
                            lhsT=xt_w[s], rhs=abf[:, sl],
                            start=True, stop=True,
                        )
                    osl = slice(q * MM_N, (q + 1) * MM_N)
                    copy_op("alt", q + (j % 2), obf[:, osl], c_ps)
                stq = nc.scalar if j % 2 == 0 else nc.sync
                stq.dma_start(out_d[s, :, j * LW:(j + 1) * LW], obf)

            # ── Emission schedule ──────────────────────────────────────
            for j in range(n_loads):
                load_cast(0, j)
                tg_chunk(0, j)
            for j in range(min(prefetch, n_loads)):
                load_cast(1, j) if S > 1 else None
            softmax(0)
            for s in range(1, S):
                ld = min(lead, n_loads)
                hb = min(holdback, n_loads - ld) if n_loads > ld else 0
                for j in range(ld):
                    stage_c_chunk(s - 1, j)
                for j in range(n_loads):
                    if j + ld < n_loads - hb:
                        stage_c_chunk(s - 1, j + ld)
                    tg_chunk(s, j)
                    if j + prefetch < n_loads:
                        load_cast(s, j + prefetch)
                if s + 1 < S:
                    for j in range(min(prefetch, n_loads)):
                        load_cast(s + 1, j)
                softmax(s)
                # Held-back C(s-1) tiles give PE work to chew on while the
                # softmax(s) DVE/ACT chain resolves (kills the phase dip).
                for j in range(n_loads - hb, n_loads):
                    stage_c_chunk(s - 1, j)
            for j in range(n_loads):
                stage_c_chunk(S - 1, j)

    nc.compile()
    return nc


_NC_CACHE: dict = {}


def _get_nc(**kw):
    key = tuple(sorted(kw.items()))
    if key not in _NC_CACHE:
        _NC_CACHE[key] = build(**kw)
    return _NC_CACHE[key]


def kernel(a, beta):
    """Full-input entry point: a [16,128,256,256] f32, beta [1] f32."""
    a = np.ascontiguousarray(np.asarray(a, dtype=np.float32))
    beta = np.asarray(beta, dtype=np.float32)
    nb, ch, h, w = a.shape
    n = h * w
    s = nb // N_CORES
    a3 = a.reshape(nb, ch, n)
    beta_b = np.broadcast_to(beta.reshape(1, 1), (ch, 1)).copy()

    nc = _get_nc(S=s, C=ch, N=n)
    in_maps = [
        {"a": a3[i * s:(i + 1) * s], "beta": beta_b} for i in range(N_CORES)
    ]
    res = run_bass_kernel_spmd(nc, in_maps, list(range(N_CORES)))
    out = np.concatenate(
        [np.asarray(res.results[i]["out"]) for i in range(N_CORES)], axis=0
    )
    return out.reshape(nb, ch, h, w).astype(np.float32)


# revision 29
# speedup vs baseline: 1.0854x; 1.0212x over previous
"""Trainium2 Bass kernel for a channel-attention block.

Reference math (per batch sample, a: [C, N] with C=128 channels,
N = H*W spatial):
    b   = a @ a.T                  # [C, C] channel affinity (Gram)
    x   = softmax(b, axis=-1)
    c   = x @ a                    # [C, N]
    out = beta * c + a

Sharding: data-parallel over the batch dim — 16 samples / 8 cores =
2 samples per NeuronCore, no cross-core communication.

Single-HBM-pass design (per sample):
  stage A: SWDGE (gpsimd) cast-DMA loads `a` in [128, LW] tiles,
           converting f32 -> bf16 in flight; the bf16 tiles stay
           RESIDENT in SBUF (16 MB/sample) so `a` is read from HBM
           exactly once and no compute engine spends time casting.
           Each tile is PE-transposed in 128-col blocks into PSUM,
           copied back to SBUF (DVE), and Gram-accumulated into one
           PSUM bank via bf16 matmuls.
  stage B: row softmax on b (DVE max, ACT exp(+bias) with fused row
           sum, DVE reciprocal). The whole affine epilogue folds into
           the stage-C weights: W = (beta/rowsum) * E + I, so
           W @ a = beta*softmax(b)@a + a IS the output — no add pass.
  stage C: c_ps = W.T.T @ a_bf16 straight from the SBUF-resident tiles
           (no second HBM read); the epilogue is a pure PSUM->SBUF
           bf16 copy, alternated between DVE and ACT; stored to HBM as
           bf16 (host upcasts to f32).

HBM traffic per core: 64 MB read (f32 a, once) + 32 MB write (bf16
out) = 96 MB. Stage C of sample s is emission-interleaved with stage A
of sample s+1 (C runs `lead` tiles ahead; SWDGE loads run `prefetch`
tiles ahead; `holdback` C tiles are re-emitted after softmax(s+1) so
PE has queued work through the phase transition). The c_ps pool keeps
4 PSUM tiles in flight so the DVE/ACT output copies pipeline instead
of serializing behind matmuls.
"""

import numpy as np

import concourse.bass as bass
import concourse.mybir as mybir
import concourse.tile as tile
from concourse import bacc
from concourse.bass_utils import run_bass_kernel_spmd
from concourse.masks import make_identity

F32 = mybir.dt.float32
BF16 = mybir.dt.bfloat16

N_CORES = 8
B, C, H, W = 16, 128, 256, 256
N_FULL = H * W
S = B // N_CORES  # samples per core


def build(S=S, C=C, N=N_FULL, LW=4096, TW=1024, MM_N=512, cache_extra=4,
          lead=3, prefetch=4, holdback=2, out_dt="bf16", eng_atcopy="dve",
          eng_cast="act", cast_split=False, ld_mode="swdge",
          stage_bufs=2, tp_bufs=3, gram_bufs=1, cps_bufs=4,
          at_bufs=6, cout_bufs=3):
    """Build + compile the per-core Bass program."""
    assert C == 128 and N % LW == 0 and LW % TW == 0 and TW % 128 == 0
    assert LW % MM_N == 0 and MM_N % 512 == 0
    assert prefetch <= lead + 1 and prefetch <= cache_extra
    nc = bacc.Bacc("TRN2", target_bir_lowering=False, debug=False)

    a_d = nc.dram_tensor("a", [S, C, N], F32, kind="ExternalInput").ap()
    beta_d = nc.dram_tensor("beta", [C, 1], F32, kind="ExternalInput").ap()
    o_dt = BF16 if out_dt == "bf16" else F32
    out_d = nc.dram_tensor("out", [S, C, N], o_dt, kind="ExternalOutput").ap()

    n_loads = N // LW
    n_chunks = LW // MM_N
    n_gram_mm = N // 128

    with tile.TileContext(nc) as tc:
        with (
            tc.tile_pool(name="const", bufs=1) as const_pool,
            tc.tile_pool(name="acache", bufs=n_loads + cache_extra) as cache_pool,
            tc.tile_pool(name="stage", bufs=stage_bufs) as stage_pool,
            tc.tile_pool(name="at", bufs=at_bufs) as at_pool,
            tc.tile_pool(name="sm", bufs=2) as sm_pool,
            tc.tile_pool(name="cout", bufs=cout_bufs) as cout_pool,
            tc.tile_pool(name="tp_ps", bufs=tp_bufs, space="PSUM") as tp_psum,
            tc.tile_pool(name="gram_ps", bufs=gram_bufs, space="PSUM") as gram_psum,
            tc.tile_pool(name="c_ps", bufs=cps_bufs, space="PSUM") as c_psum,
        ):
            ident_bf = const_pool.tile([128, 128], BF16, tag="identbf")
            make_identity(nc, ident_bf)
            beta_sb = const_pool.tile([C, 1], F32, tag="beta")
            nc.sync.dma_start(beta_sb, beta_d)

            def copy_op(engine_sel, idx, out, in_):
                """Route a copy/cast to ACT or DVE per engine_sel."""
                if engine_sel == "act" or (engine_sel == "alt" and idx % 2 == 0):
                    nc.scalar.copy(out, in_)
                else:
                    nc.vector.tensor_copy(out, in_)

            gram_state = {}   # s -> [b_ps, mm_count]
            xt_w = {}         # s -> beta-scaled lhsT weights for stage C
            cached = {}       # (s, j) -> SBUF-resident bf16 a tile

            def load_cast(s, j):
                """HWDGE f32 load into staging; ACT/DVE cast into cache
                (or SWDGE cast-in-DMA when ld_mode='swdge')."""
                abf = cache_pool.tile([C, LW], BF16, tag="acache",
                                      name=f"ac_{s}_{j}")
                cached[(s, j)] = abf
                src = a_d[s, :, j * LW:(j + 1) * LW]
                # headsplit: in sample 0 (the head phase) the HWDGE rings
                # and ACT are otherwise idle, and SWDGE cast-DMA alone paces
                # the head at half line-rate. So odd head tiles load f32 via
                # HWDGE + ACT cast while even tiles use SWDGE — the two paths
                # split the head's DMA time. Every later sample is SWDGE-only
                # (zero engine cost, keeps ACT/DVE free for the epilogue).
                use_swdge = (
                    ld_mode == "swdge"
                    or (ld_mode == "hybrid" and s > 0)
                    or (ld_mode == "headsplit" and (s > 0 or j % 2 == 0))
                )
                if use_swdge:
                    nc.gpsimd.dma_start(abf, src)
                    return
                stg = stage_pool.tile([C, LW], F32, tag="stage",
                                      name=f"stg_{s}_{j}")
                ld = nc.sync if j % 4 == 1 else nc.scalar
                ld.dma_start(stg, src)
                if cast_split:
                    hw = LW // 2
                    nc.scalar.copy(abf[:, :hw], stg[:, :hw])
                    nc.vector.tensor_copy(abf[:, hw:], stg[:, hw:])
                else:
                    copy_op(eng_cast, j, abf, stg)

            def tg_chunk(s, j):
                """PE-transpose tile j of sample s and Gram-accumulate."""
                if s not in gram_state:
                    b_ps = gram_psum.tile([C, C], F32, tag="gram",
                                          name=f"gram_{s}")
                    gram_state[s] = [b_ps, 0]
                st = gram_state[s]
                b_ps = st[0]
                abf = cached[(s, j)]
                for g in range(LW // TW):
                    tp = tp_psum.tile([128, TW], BF16, tag="tp",
                                      name=f"tp_{s}_{j}_{g}")
                    for q in range(TW // 128):
                        nc.tensor.transpose(
                            tp[:, q * 128:(q + 1) * 128],
                            abf[:, g * TW + q * 128:g * TW + (q + 1) * 128],
                            ident_bf,
                        )
                    at_t = at_pool.tile([128, TW], BF16, tag="at",
                                        name=f"at_{s}_{j}_{g}")
                    copy_op(eng_atcopy, g, at_t, tp)
                    for q in range(TW // 128):
                        st[1] += 1
                        nc.tensor.matmul(
                            b_ps,
                            lhsT=at_t[:, q * 128:(q + 1) * 128],
                            rhs=at_t[:, q * 128:(q + 1) * 128],
                            start=(st[1] == 1),
                            stop=(st[1] == n_gram_mm),
                        )

            def softmax(s):
                """Fold the whole affine epilogue into the stage-C
                weights: W = (beta/rowsum) * exp(b - rowmax) + I, so
                W @ a = beta*softmax(b)@a + a = out."""
                b_ps = gram_state[s][0]
                negm = sm_pool.tile([C, 1], F32, tag="negm", name=f"negm_{s}")
                nc.vector.tensor_reduce(
                    negm, b_ps, axis=mybir.AxisListType.X,
                    op=mybir.AluOpType.max, negate=True,
                )
                e_t = sm_pool.tile([C, C], BF16, tag="e", name=f"e_{s}")
                ssum = sm_pool.tile([C, 1], F32, tag="ssum", name=f"ssum_{s}")
                nc.scalar.activation(
                    e_t, b_ps, mybir.ActivationFunctionType.Exp,
                    bias=negm, accum_out=ssum,
                )
                rec = sm_pool.tile([C, 1], F32, tag="rec", name=f"rec_{s}")
                nc.vector.reciprocal(rec, ssum)
                bs = sm_pool.tile([C, 1], F32, tag="bs", name=f"bs_{s}")
                nc.vector.tensor_scalar_mul(bs, rec, beta_sb)
                e_s = sm_pool.tile([C, C], BF16, tag="es", name=f"es_{s}")
                nc.vector.tensor_scalar_mul(e_s, e_t, bs)
                e_w = sm_pool.tile([C, C], BF16, tag="ew", name=f"ew_{s}")
                nc.vector.tensor_tensor(
                    out=e_w, in0=e_s, in1=ident_bf, op=mybir.AluOpType.add,
                )
                xt_ps = tp_psum.tile([128, TW], BF16, tag="tp", name=f"xtp_{s}")
                nc.tensor.transpose(xt_ps[:, :128], e_w, ident_bf)
                xt_sb = sm_pool.tile([C, C], BF16, tag="xt", name=f"xt_{s}")
                nc.scalar.copy(xt_sb, xt_ps[:, :128])
                xt_w[s] = xt_sb

            def stage_c_chunk(s, j):
                """c_ps = W @ a_bf16 from SBUF (the full output); the
                epilogue is a pure PSUM->SBUF bf16 copy (DVE/ACT alt)."""
                abf = cached.pop((s, j))
                obf = cout_pool.tile([C, LW], o_dt, tag="cout",
                                     name=f"cout_{s}_{j}")
                for q in range(n_chunks):
                    c_ps = c_psum.tile([128, MM_N], F32, tag="cps",
                                       name=f"cps_{s}_{j}_{q}")
                    for h in range(MM_N // 512):
                        sl = slice(q * MM_N + h * 512, q * MM_N + (h + 1) * 512)
                        nc.tensor.matmul(
                            c_ps[:, h * 512:(h + 1) * 512],
                            lhsT=xt_w[s], rhs=abf[:, sl],
                            start=True, stop=True,
                        )
                    osl = slice(q * MM_N, (q + 1) * MM_N)
                    copy_op("alt", q + (j % 2), obf[:, osl], c_ps)
                stq = nc.scalar if j % 2 == 0 else nc.sync
                stq.dma_start(out_d[s, :, j * LW:(j + 1) * LW], obf)

            # ── Emission schedule ──────────────────────────────────────
            for j in range(n_loads):
                load_cast(0, j)
                tg_chunk(0, j)
            for j in range(min(prefetch, n_loads)):
                load_cast(1, j) if S > 1 else None
            softmax(0)
            for s in range(1, S):
                ld = min(lead, n_loads)
                hb = min(holdback, n_loads - ld) if n_loads > ld else 0
                for j in range(ld):
                    stage_c_chunk(s - 1, j)
                for j in range(n_loads):
                    if j + ld < n_loads - hb:
                        stage_c_chunk(s - 1, j + ld)
                    tg_chunk(s, j)
                    if j + prefetch < n_loads:
                        load_cast(s, j + prefetch)
                if s + 1 < S:
                    for j in range(min(prefetch, n_loads)):
                        load_cast(s + 1, j)
                softmax(s)
                # Held-back C(s-1) tiles give PE work to chew on while the
                # softmax(s) DVE/ACT chain resolves (kills the phase dip).
                for j in range(n_loads - hb, n_loads):
                    stage_c_chunk(s - 1, j)
            for j in range(n_loads):
                stage_c_chunk(S - 1, j)

    nc.compile()
    return nc


_NC_CACHE: dict = {}


def _get_nc(**kw):
    key = tuple(sorted(kw.items()))
    if key not in _NC_CACHE:
        _NC_CACHE[key] = build(**kw)
    return _NC_CACHE[key]


def kernel(a, beta):
    """Full-input entry point: a [16,128,256,256] f32, beta [1] f32."""
    a = np.ascontiguousarray(np.asarray(a, dtype=np.float32))
    beta = np.asarray(beta, dtype=np.float32)
    nb, ch, h, w = a.shape
    n = h * w
    s = nb // N_CORES
    a3 = a.reshape(nb, ch, n)
    beta_b = np.broadcast_to(beta.reshape(1, 1), (ch, 1)).copy()

    nc = _get_nc(S=s, C=ch, N=n)
    in_maps = [
        {"a": a3[i * s:(i + 1) * s], "beta": beta_b} for i in range(N_CORES)
    ]
    res = run_bass_kernel_spmd(nc, in_maps, list(range(N_CORES)))
    out = np.concatenate(
        [np.asarray(res.results[i]["out"]) for i in range(N_CORES)], axis=0
    )
    return out.reshape(nb, ch, h, w).astype(np.float32)


# revision 30
# speedup vs baseline: 1.1285x; 1.0397x over previous
"""Trainium2 Bass kernel for a channel-attention block.

Reference math (per batch sample, a: [C, N] with C=128 channels,
N = H*W spatial):
    b   = a @ a.T                  # [C, C] channel affinity (Gram)
    x   = softmax(b, axis=-1)
    c   = x @ a                    # [C, N]
    out = beta * c + a

Sharding: data-parallel over the batch dim — 16 samples / 8 cores =
2 samples per NeuronCore, no cross-core communication.

Single-HBM-pass design (per sample):
  stage A: SWDGE (gpsimd) cast-DMA loads `a` in [128, LW] tiles,
           converting f32 -> bf16 in flight; the bf16 tiles stay
           RESIDENT in SBUF (16 MB/sample) so `a` is read from HBM
           exactly once and no compute engine spends time casting.
           Each tile is PE-transposed in 128-col blocks into PSUM,
           copied back to SBUF (DVE), and Gram-accumulated into one
           PSUM bank via bf16 matmuls.
  stage B: row softmax on b (DVE max, ACT exp(+bias) with fused row
           sum, DVE reciprocal). The whole affine epilogue folds into
           the stage-C weights: W = (beta/rowsum) * E + I, so
           W @ a = beta*softmax(b)@a + a IS the output — no add pass.
  stage C: c_ps = W.T.T @ a_bf16 straight from the SBUF-resident tiles
           (no second HBM read); the epilogue is a pure PSUM->SBUF
           bf16 copy, alternated between DVE and ACT; stored to HBM as
           bf16 (host upcasts to f32).

HBM traffic per core: 64 MB read (f32 a, once) + 32 MB write (bf16
out) = 96 MB. Stage C of sample s is emission-interleaved with stage A
of sample s+1 (C runs `lead` tiles ahead; SWDGE loads run `prefetch`
tiles ahead; `holdback` C tiles are re-emitted after softmax(s+1) so
PE has queued work through the phase transition). The c_ps pool keeps
4 PSUM tiles in flight so the DVE/ACT output copies pipeline instead
of serializing behind matmuls.
"""

import numpy as np

import concourse.bass as bass
import concourse.mybir as mybir
import concourse.tile as tile
from concourse import bacc
from concourse.bass_utils import run_bass_kernel_spmd
from concourse.masks import make_identity

F32 = mybir.dt.float32
BF16 = mybir.dt.bfloat16

N_CORES = 8
B, C, H, W = 16, 128, 256, 256
N_FULL = H * W
S = B // N_CORES  # samples per core


def build(S=S, C=C, N=N_FULL, LW=4096, TW=1024, MM_N=512, cache_extra=4,
          lead=3, prefetch=4, holdback=2, out_dt="bf16", eng_atcopy="dve",
          eng_cast="act", cast_split=False, ld_mode="swdge",
          stage_bufs=2, tp_bufs=3, gram_bufs=1, cps_bufs=4,
          at_bufs=4, cout_bufs=3):
    """Build + compile the per-core Bass program."""
    assert C == 128 and N % LW == 0 and LW % TW == 0 and TW % 128 == 0
    assert LW % MM_N == 0 and MM_N % 512 == 0
    assert prefetch <= lead + 1 and prefetch <= cache_extra
    nc = bacc.Bacc("TRN2", target_bir_lowering=False, debug=False)

    a_d = nc.dram_tensor("a", [S, C, N], F32, kind="ExternalInput").ap()
    beta_d = nc.dram_tensor("beta", [C, 1], F32, kind="ExternalInput").ap()
    o_dt = BF16 if out_dt == "bf16" else F32
    out_d = nc.dram_tensor("out", [S, C, N], o_dt, kind="ExternalOutput").ap()

    n_loads = N // LW
    n_chunks = LW // MM_N
    n_gram_mm = N // 128

    with tile.TileContext(nc) as tc:
        with (
            tc.tile_pool(name="const", bufs=1) as const_pool,
            tc.tile_pool(name="acache", bufs=n_loads + cache_extra) as cache_pool,
            tc.tile_pool(name="stage", bufs=stage_bufs) as stage_pool,
            tc.tile_pool(name="at", bufs=at_bufs) as at_pool,
            tc.tile_pool(name="sm", bufs=2) as sm_pool,
            tc.tile_pool(name="cout", bufs=cout_bufs) as cout_pool,
            tc.tile_pool(name="tp_ps", bufs=tp_bufs, space="PSUM") as tp_psum,
            tc.tile_pool(name="gram_ps", bufs=gram_bufs, space="PSUM") as gram_psum,
            tc.tile_pool(name="c_ps", bufs=cps_bufs, space="PSUM") as c_psum,
        ):
            ident_bf = const_pool.tile([128, 128], BF16, tag="identbf")
            make_identity(nc, ident_bf)
            beta_sb = const_pool.tile([C, 1], F32, tag="beta")
            nc.sync.dma_start(beta_sb, beta_d)

            def copy_op(engine_sel, idx, out, in_):
                """Route a copy/cast to ACT or DVE per engine_sel."""
                if engine_sel == "act" or (engine_sel == "alt" and idx % 2 == 0):
                    nc.scalar.copy(out, in_)
                else:
                    nc.vector.tensor_copy(out, in_)

            gram_state = {}   # s -> [b_ps, mm_count]
            xt_w = {}         # s -> beta-scaled lhsT weights for stage C
            cached = {}       # (s, j) -> SBUF-resident bf16 a tile

            def load_cast(s, j):
                """HWDGE f32 load into staging; ACT/DVE cast into cache
                (or SWDGE cast-in-DMA when ld_mode='swdge')."""
                abf = cache_pool.tile([C, LW], BF16, tag="acache",
                                      name=f"ac_{s}_{j}")
                cached[(s, j)] = abf
                src = a_d[s, :, j * LW:(j + 1) * LW]
                # headsplit: in sample 0 (the head phase) the HWDGE rings
                # and ACT are otherwise idle, and SWDGE cast-DMA alone paces
                # the head at half line-rate. So odd head tiles load f32 via
                # HWDGE + ACT cast while even tiles use SWDGE — the two paths
                # split the head's DMA time. Every later sample is SWDGE-only
                # (zero engine cost, keeps ACT/DVE free for the epilogue).
                use_swdge = (
                    ld_mode == "swdge"
                    or (ld_mode == "hybrid" and s > 0)
                    or (ld_mode == "headsplit" and (s > 0 or j % 2 == 0))
                )
                if use_swdge:
                    nc.gpsimd.dma_start(abf, src)
                    return
                stg = stage_pool.tile([C, LW], F32, tag="stage",
                                      name=f"stg_{s}_{j}")
                ld = nc.sync if j % 4 == 1 else nc.scalar
                ld.dma_start(stg, src)
                if cast_split:
                    hw = LW // 2
                    nc.scalar.copy(abf[:, :hw], stg[:, :hw])
                    nc.vector.tensor_copy(abf[:, hw:], stg[:, hw:])
                else:
                    copy_op(eng_cast, j, abf, stg)

            def tg_chunk(s, j):
                """PE-transpose tile j of sample s and Gram-accumulate."""
                if s not in gram_state:
                    b_ps = gram_psum.tile([C, C], F32, tag="gram",
                                          name=f"gram_{s}")
                    gram_state[s] = [b_ps, 0]
                st = gram_state[s]
                b_ps = st[0]
                abf = cached[(s, j)]
                for g in range(LW // TW):
                    tp = tp_psum.tile([128, TW], BF16, tag="tp",
                                      name=f"tp_{s}_{j}_{g}")
                    for q in range(TW // 128):
                        nc.tensor.transpose(
                            tp[:, q * 128:(q + 1) * 128],
                            abf[:, g * TW + q * 128:g * TW + (q + 1) * 128],
                            ident_bf,
                        )
                    at_t = at_pool.tile([128, TW], BF16, tag="at",
                                        name=f"at_{s}_{j}_{g}")
                    copy_op(eng_atcopy, g, at_t, tp)
                    for q in range(TW // 128):
                        st[1] += 1
                        nc.tensor.matmul(
                            b_ps,
                            lhsT=at_t[:, q * 128:(q + 1) * 128],
                            rhs=at_t[:, q * 128:(q + 1) * 128],
                            start=(st[1] == 1),
                            stop=(st[1] == n_gram_mm),
                        )

            def softmax(s):
                """Fold the whole affine epilogue into the stage-C
                weights: W = (beta/rowsum) * exp(b - rowmax) + I, so
                W @ a = beta*softmax(b)@a + a = out."""
                b_ps = gram_state[s][0]
                negm = sm_pool.tile([C, 1], F32, tag="negm", name=f"negm_{s}")
                nc.vector.tensor_reduce(
                    negm, b_ps, axis=mybir.AxisListType.X,
                    op=mybir.AluOpType.max, negate=True,
                )
                e_t = sm_pool.tile([C, C], BF16, tag="e", name=f"e_{s}")
                ssum = sm_pool.tile([C, 1], F32, tag="ssum", name=f"ssum_{s}")
                nc.scalar.activation(
                    e_t, b_ps, mybir.ActivationFunctionType.Exp,
                    bias=negm, accum_out=ssum,
                )
                rec = sm_pool.tile([C, 1], F32, tag="rec", name=f"rec_{s}")
                nc.vector.reciprocal(rec, ssum)
                bs = sm_pool.tile([C, 1], F32, tag="bs", name=f"bs_{s}")
                nc.vector.tensor_scalar_mul(bs, rec, beta_sb)
                e_s = sm_pool.tile([C, C], BF16, tag="es", name=f"es_{s}")
                nc.vector.tensor_scalar_mul(e_s, e_t, bs)
                e_w = sm_pool.tile([C, C], BF16, tag="ew", name=f"ew_{s}")
                nc.vector.tensor_tensor(
                    out=e_w, in0=e_s, in1=ident_bf, op=mybir.AluOpType.add,
                )
                xt_ps = tp_psum.tile([128, TW], BF16, tag="tp", name=f"xtp_{s}")
                nc.tensor.transpose(xt_ps[:, :128], e_w, ident_bf)
                xt_sb = sm_pool.tile([C, C], BF16, tag="xt", name=f"xt_{s}")
                nc.scalar.copy(xt_sb, xt_ps[:, :128])
                xt_w[s] = xt_sb

            def stage_c_chunk(s, j):
                """c_ps = W @ a_bf16 from SBUF (the full output); the
                epilogue is a pure PSUM->SBUF bf16 copy (DVE/ACT alt)."""
                abf = cached.pop((s, j))
                obf = cout_pool.tile([C, LW], o_dt, tag="cout",
                                     name=f"cout_{s}_{j}")
                for q in range(n_chunks):
                    c_ps = c_psum.tile([128, MM_N], F32, tag="cps",
                                       name=f"cps_{s}_{j}_{q}")
                    for h in range(MM_N // 512):
                        sl = slice(q * MM_N + h * 512, q * MM_N + (h + 1) * 512)
                        nc.tensor.matmul(
                            c_ps[:, h * 512:(h + 1) * 512],
                            lhsT=xt_w[s], rhs=abf[:, sl],
                            start=True, stop=True,
                        )
                    osl = slice(q * MM_N, (q + 1) * MM_N)
                    copy_op("alt", q + (j % 2), obf[:, osl], c_ps)
                stq = nc.scalar if j % 2 == 0 else nc.sync
                stq.dma_start(out_d[s, :, j * LW:(j + 1) * LW], obf)

            # ── Emission schedule ──────────────────────────────────────
            for j in range(n_loads):
                load_cast(0, j)
                tg_chunk(0, j)
            for j in range(min(prefetch, n_loads)):
                load_cast(1, j) if S > 1 else None
            softmax(0)
            for s in range(1, S):
                ld = min(lead, n_loads)
                hb = min(holdback, n_loads - ld) if n_loads > ld else 0
                for j in range(ld):
                    stage_c_chunk(s - 1, j)
                for j in range(n_loads):
                    if j + ld < n_loads - hb:
                        stage_c_chunk(s - 1, j + ld)
                    tg_chunk(s, j)
                    if j + prefetch < n_loads:
                        load_cast(s, j + prefetch)
                if s + 1 < S:
                    for j in range(min(prefetch, n_loads)):
                        load_cast(s + 1, j)
                softmax(s)
                # Held-back C(s-1) tiles give PE work to chew on while the
                # softmax(s) DVE/ACT chain resolves (kills the phase dip).
                for j in range(n_loads - hb, n_loads):
                    stage_c_chunk(s - 1, j)
            for j in range(n_loads):
                stage_c_chunk(S - 1, j)

    nc.compile()
    return nc


_NC_CACHE: dict = {}


def _get_nc(**kw):
    key = tuple(sorted(kw.items()))
    if key not in _NC_CACHE:
        _NC_CACHE[key] = build(**kw)
    return _NC_CACHE[key]


def kernel(a, beta):
    """Full-input entry point: a [16,128,256,256] f32, beta [1] f32."""
    a = np.ascontiguousarray(np.asarray(a, dtype=np.float32))
    beta = np.asarray(beta, dtype=np.float32)
    nb, ch, h, w = a.shape
    n = h * w
    s = nb // N_CORES
    a3 = a.reshape(nb, ch, n)
    beta_b = np.broadcast_to(beta.reshape(1, 1), (ch, 1)).copy()

    nc = _get_nc(S=s, C=ch, N=n)
    in_maps = [
        {"a": a3[i * s:(i + 1) * s], "beta": beta_b} for i in range(N_CORES)
    ]
    res = run_bass_kernel_spmd(nc, in_maps, list(range(N_CORES)))
    out = np.concatenate(
        [np.asarray(res.results[i]["out"]) for i in range(N_CORES)], axis=0
    )
    return out.reshape(nb, ch, h, w).astype(np.float32)
